# revision 3
# baseline (speedup 1.0000x reference)
"""Multi-head attention forward (a=4, m=t=2048, e=1024, 16 heads x 64) on 8 NeuronCores.

Sharding: 8 shards = 4 batches x 2 head-groups (data + head/tensor parallel).
Each core handles one (batch, head-group): its 8 heads' QKV projections
(column-sharded weights) and attention, producing out[b, :, g*512:(g+1)*512].

Per-core kernel (all matmuls fp32r = full-rate ~TF32 precision):
  Phase A: qp^T = Wq_g @ q^T  [512, 2048]   (e-contraction, PSUM accum)
           kp^T = Wk_g @ k^T  [512, 2048]
           vp   = (v @ Wv_g^T | 1) [2048, 8*65]  (ones-augmented per head)
  Phase B (per head, per 512-wide m-chunk, flash-style over key tiles):
           S^T[t,m] = K_h @ Q_h^T   (PSUM superblocks of 2-3 key tiles)
           P^T = exp(S^T)           (one ACT pass per superblock, no max-sub:
                                     |S| < ~60 << 88 so fp32 exp is safe)
           [O'; sums]^T += [V_h|1]^T @ P^T  (PSUM accum over 16 key tiles)
           out^T = O'^T * (1/sums)  (DVE recip + gpsimd row-broadcast + DVE mul)
Host: transpose/concat the 8 outT shards into [4, 2048, 1024].
"""
import numpy as np

A, M, T, E = 4, 2048, 2048, 1024
H, C = 16, 64
NCORES = 8
NG = 2                 # head groups (tensor-parallel dim)
NH = H // NG           # heads per core
N = NH * C             # 512: per-core projection width
ET = E // 128          # 8 e-tiles
NT = N // 128          # 4 n-tiles
MCH = M // 512         # 4 m-chunks
TTI = T // 128         # 16 key tiles
SUPERS = [(0, 3), (3, 6), (6, 9), (9, 12), (12, 14), (14, 16)]

_nc = None


def _build():
    import concourse.mybir as mybir
    import concourse.tile as tile
    import concourse.bacc as bacc

    f32, f32r = mybir.dt.float32, mybir.dt.float32r
    nc = bacc.Bacc(None, target_bir_lowering=False)

    qT = nc.dram_tensor("qT", [E, M], f32r, kind="ExternalInput")
    kT = nc.dram_tensor("kT", [E, T], f32r, kind="ExternalInput")
    vT = nc.dram_tensor("vT", [E, T], f32r, kind="ExternalInput")
    wqT = nc.dram_tensor("wqT", [E, N], f32r, kind="ExternalInput")
    wkT = nc.dram_tensor("wkT", [E, N], f32r, kind="ExternalInput")
    wvT = nc.dram_tensor("wvT", [E, N], f32r, kind="ExternalInput")
    bqn = nc.dram_tensor("bqn", [128, NT], f32, kind="ExternalInput")
    bkn = nc.dram_tensor("bkn", [128, NT], f32, kind="ExternalInput")
    bvr = nc.dram_tensor("bvr", [1, N], f32, kind="ExternalInput")
    onesd = nc.dram_tensor("onesd", [128, NH], f32r, kind="ExternalInput")
    outT = nc.dram_tensor("outT", [N, M], f32, kind="ExternalOutput")

    Exp = mybir.ActivationFunctionType.Exp

    with tile.TileContext(nc) as tc:
        with tc.tile_pool(name="resident", bufs=1) as res, \
             tc.tile_pool(name="inT", bufs=ET) as in_pool, \
             tc.tile_pool(name="w", bufs=ET) as w_pool:

            # small constants
            bq_sb = res.tile([128, NT], f32, tag="bq")
            bk_sb = res.tile([128, NT], f32, tag="bk")
            bvr_sb = res.tile([1, N], f32, tag="bvr")
            ones_sb = res.tile([128, NH], f32r, tag="ones1")
            nc.sync.dma_start(bq_sb[:], bqn[:])
            nc.sync.dma_start(bk_sb[:], bkn[:])
            nc.sync.dma_start(bvr_sb[:], bvr[:])
            nc.sync.dma_start(ones_sb[:], onesd[:])
            bv_bc = res.tile([128, N], f32, tag="bvbc")
            nc.gpsimd.partition_broadcast(bv_bc[:], bvr_sb[:])

            # resident activation tiles
            qp_sb = [res.tile([128, M], f32r, tag=f"qp{i}", name=f"qp{i}") for i in range(NT)]
            kp_sb = [res.tile([128, T], f32r, tag=f"kp{i}", name=f"kp{i}") for i in range(NT)]
            vpr = [res.tile([128, NH, C + 1], f32r, tag=f"vpr{i}", name=f"vpr{i}") for i in range(TTI)]

            # ---------------- Phase A: projections ----------------
            # input tiles streamed in m/t halves of [128, 1024] to fit SBUF
            with tc.tile_pool(name="pp", bufs=6, space="PSUM") as pp:
                for xT_d, wT_d, xp, b_sb in ((qT, wqT, qp_sb, bq_sb),
                                             (kT, wkT, kp_sb, bk_sb)):
                    wt = []
                    for e in range(ET):
                        w = w_pool.tile([128, N], f32r, tag="w", name="w")
                        nc.sync.dma_start(w[:], wT_d[e * 128:(e + 1) * 128, :])
                        wt.append(w)
                    for mh in range(2):
                        hsl = slice(mh * 1024, (mh + 1) * 1024)
                        xt = []
                        for e in range(ET):
                            x = in_pool.tile([128, 1024], f32r, tag="in", name="xin")
                            nc.sync.dma_start(x[:], xT_d[e * 128:(e + 1) * 128, hsl])
                            xt.append(x)
                        for mci in range(2):
                            mc = mh * 2 + mci
                            msl = slice(mc * 512, (mc + 1) * 512)
                            lsl = slice(mci * 512, (mci + 1) * 512)
                            for n in range(NT):
                                ps = pp.tile([128, 512], f32, tag="pp")
                                for e in range(ET):
                                    nc.tensor.matmul(
                                        ps[:], wt[e][:, n * 128:(n + 1) * 128],
                                        xt[e][:, lsl],
                                        start=(e == 0), stop=(e == ET - 1))
                                nc.vector.tensor_scalar_add(
                                    xp[n][:, msl], ps[:], b_sb[:, n:n + 1])

                # V projection: vp[t, n] with ones column per head
                wt = []
                for e in range(ET):
                    w = w_pool.tile([128, N], f32r, tag="w", name="w")
                    nc.sync.dma_start(w[:], wvT[e * 128:(e + 1) * 128, :])
                    wt.append(w)
                for th in range(2):
                    hsl = slice(th * 1024, (th + 1) * 1024)
                    vt = []
                    for e in range(ET):
                        x = in_pool.tile([128, 1024], f32r, tag="in", name="xin")
                        nc.sync.dma_start(x[:], vT[e * 128:(e + 1) * 128, hsl])
                        vt.append(x)
                    for ti in range(TTI // 2):
                        t = th * (TTI // 2) + ti
                        ps = pp.tile([128, 512], f32, tag="pp")
                        for e in range(ET):
                            nc.tensor.matmul(
                                ps[:], vt[e][:, ti * 128:(ti + 1) * 128], wt[e][:],
                                start=(e == 0), stop=(e == ET - 1))
                        nc.sync.dma_start(
                            vpr[t][:, :, C:C + 1],
                            onesd.rearrange("p (a o) -> p a o", o=1))
                        nc.vector.tensor_add(
                            vpr[t][:, :, 0:C],
                            ps[:].rearrange("p (h c) -> p h c", c=C),
                            bv_bc[:].rearrange("p (h c) -> p h c", c=C))

            # ---------------- Phase B: attention ----------------
            with tc.tile_pool(name="st", bufs=2, space="PSUM") as stp, \
                 tc.tile_pool(name="op", bufs=2, space="PSUM") as opp, \
                 tc.tile_pool(name="pt", bufs=3) as ptp, \
                 tc.tile_pool(name="nrm", bufs=2) as nrm:
                for h in range(NH):
                    po = (h % 2) * C
                    qh = qp_sb[h // 2][po:po + C, :]
                    kh = kp_sb[h // 2][po:po + C, :]
                    for mc in range(MCH):
                        msl = slice(mc * 512, (mc + 1) * 512)
                        op_ps = opp.tile([C + 1, 512], f32, tag="op")
                        for (t0, t1) in SUPERS:
                            L = t1 - t0
                            st = stp.tile([128, 3, 512], f32, tag="st")
                            for i in range(L):
                                t = t0 + i
                                nc.tensor.matmul(
                                    st[:, i, :], kh[:, t * 128:(t + 1) * 128],
                                    qh[:, msl], start=True, stop=True)
                            pt = ptp.tile([128, 3, 512], f32r, tag="pt")
                            nc.scalar.activation(pt[:, 0:L, :], st[:, 0:L, :], Exp)
                            for i in range(L):
                                t = t0 + i
                                nc.tensor.matmul(
                                    op_ps[:], vpr[t][:, h, :], pt[:, i, :],
                                    start=(t == 0), stop=(t == TTI - 1))
                        recip = nrm.tile([1, 512], f32, tag="recip")
                        nc.vector.reciprocal(recip[:], op_ps[C:C + 1, :])
                        bc = nrm.tile([C, 512], f32, tag="bc")
                        nc.gpsimd.partition_broadcast(bc[:], recip[:])
                        ot = nrm.tile([C, 512], f32, tag="ot")
                        nc.vector.tensor_mul(ot[:], op_ps[0:C, :], bc[:])
                        nc.sync.dma_start(outT[h * C:(h + 1) * C, msl], ot[:])

    nc.compile()
    return nc


def _get_nc():
    global _nc
    if _nc is None:
        _nc = _build()
    return _nc


def kernel(q, k, v, Wq, bq, Wk, bk, Wv, bv):
    from concourse.bass_utils import run_bass_kernel_spmd

    q = np.asarray(q, np.float32)
    k = np.asarray(k, np.float32)
    v = np.asarray(v, np.float32)
    Wq = np.asarray(Wq, np.float32)
    Wk = np.asarray(Wk, np.float32)
    Wv = np.asarray(Wv, np.float32)
    bq = np.asarray(bq, np.float32)
    bk = np.asarray(bk, np.float32)
    bv = np.asarray(bv, np.float32)

    nc = _get_nc()
    ones = np.ones((128, NH), np.float32)
    in_maps = []
    for d in range(NCORES):
        b, g = d // NG, d % NG
        sl = slice(g * N, (g + 1) * N)
        in_maps.append({
            "qT": np.ascontiguousarray(q[b].T),
            "kT": np.ascontiguousarray(k[b].T),
            "vT": np.ascontiguousarray(v[b].T),
            "wqT": np.ascontiguousarray(Wq[sl, :].T),
            "wkT": np.ascontiguousarray(Wk[sl, :].T),
            "wvT": np.ascontiguousarray(Wv[sl, :].T),
            "bqn": np.ascontiguousarray(bq[sl].reshape(NT, 128).T),
            "bkn": np.ascontiguousarray(bk[sl].reshape(NT, 128).T),
            "bvr": np.ascontiguousarray(bv[sl].reshape(1, N)),
            "onesd": ones,
        })

    last_err = None
    for _ in range(3):
        try:
            res = run_bass_kernel_spmd(nc, in_maps, core_ids=list(range(NCORES)))
            break
        except Exception as e:  # transient NRT device wedges: retry
            last_err = e
    else:
        raise last_err

    out = np.empty((A, M, E), np.float32)
    for d in range(NCORES):
        b, g = d // NG, d % NG
        out[b, :, g * N:(g + 1) * N] = res.results[d]["outT"].T
    return out
